# revision 1
# baseline (speedup 1.0000x reference)
"""BiLIF (bidirectional leaky-integrate-and-fire) node on 8 Trainium2 NeuronCores.

Problem: inputs [T=16, B=64, N=65536] f32.
  s1 = LIF-scan(x,          tau=4/3, v_th=0.75)   (hard reset to 0)
  s2 = LIF-scan(flip(x, 0), tau=4/3, v_th=1.25)
  out = (s1 + s2) / 2

Strategy
  - Shard the batch dim across the 8 cores (pure data parallel). Per core:
    8*65536 positions = 128 partitions x 4096 columns, two [128, 2048]
    column chunks. Both direction scans run concurrently: at step t the
    forward scan consumes x[t], the backward scan consumes x[15-t], so
    out[t] completes at step t and every x tile is loaded exactly once.
  - DVE does ONLY the two fused LIF step passes (charge+reset as one
    2-src custom op per direction per step -- the irreducible chain).
    The t=0 charges run on ACT (Copy with scale/bias) to keep DVE lean.
  - Direction 2 keeps a SHIFTED state g = h2 - 0.5 (the shift is folded
    into the custom op's three constants), so both directions spike at
    the SAME threshold 0.75. h1 and g share one [128, 4096] tile and ONE
    ACT Sign instruction produces both sigma tiles in fp8e4m3 (exact on
    {-1,0,1}; fp8 halves the ACT-write + PE-read SBUF traffic, which
    was contending with DVE's fp32 streams).
  - PE combines AND packs: 8 accumulating 512-col matmuls per chunk-step
    (one PSUM bank each; >512 fp32 psum cols per matmul fails the ISA
    num_elements check). Weights map partition pair (2q, 2q+1) -> psum
    row q (strip A, data cols 0:1024) or 64+q (strip B, cols 1024:2048)
    with weights (0.5, 1.5): psum = t0 + 3*t1, t = (sig1+sig2)/2 in
    {-1,0,1}, |p| <= 4 in 0.5 steps -- exact in fp8e4m3. 2 data columns
    pack into one fp8 byte -> output DMA is 0.5 B/elem (4.2 MB/core).
  - ACT drains psum -> fp8 one step LATE, and the drain is emitted
    BEFORE the sign so the in-order ACT queue never stalls on DVE/PE.
  - Host decodes balanced ternary: t1 = round(p/3), t0 = p - 3*t1,
    out = (t + 1)/2.
  - x-tile pool holds 17 buffers (16 live tiles per chunk + 1 spare) so
    the next chunk's loads start during the current chunk's tail steps,
    and PSUM uses all 8 banks (4 x 2-bank tiles) to decouple PE from
    the ACT drain. This boundary prefetch was worth ~14 us/rep.
  Measured (R=17 burst differencing, same harness throughout):
  84.9 us/rep, vs 99.0 us before the prefetch/psum change, 110.9 us for
  the bf16-sigma variant, and 139.0 us for the all-DVE baseline.
"""

import numpy as np
import ml_dtypes  # noqa: F401

import concourse.bacc as bacc
import concourse.mybir as mybir
import concourse.tile as tile
import concourse.dve_ops as dve_ops
from concourse.dve_ops import DveOp
from concourse.dve_spec import (
    C0,
    C1,
    C2,
    Spec,
    Src0,
    Src1,
    Zero,
    _has_src1,
    lower,
    select,
)
from concourse.dve_uop import DveOpSpec
from concourse import bass_utils

T, B, N = 16, 64, 65536
NCORES = 8
BS = B // NCORES        # batch rows per core
POS = BS * N            # independent positions per core
P = 128
FREE = POS // P         # 4096 columns per partition
CHUNK = 2048
NCHUNK = FREE // CHUNK
HALF = CHUNK // 2       # 1024: packed output columns per chunk
R = 0.75                # fl32(1 / fl32(4/3)) == 0.75 exactly
TH1, TH2 = 0.75, 1.25
SHIFT = TH2 - TH1       # dir-2 state kept as g = h2 - SHIFT
F32 = mybir.dt.float32
BF16 = mybir.dt.bfloat16
FP8 = mybir.dt.float8e4
AF = mybir.ActivationFunctionType


def _register(name: str, spec: Spec) -> DveOp:
    """Register a custom DVE op at runtime (uops sha computed here)."""
    if name in dve_ops._SUB_OPCODE_FOR_NAME:
        for op in dve_ops.OPS:
            if op.name == name:
                return op
    row = dve_ops._CUSTOM_DVE_ROW_BASE + len(dve_ops.OPS)
    assert row < 0x20, "custom DVE opcode rows exhausted"
    sha = {}
    for ver in ("v3", "v4"):
        s = DveOpSpec(name=name, opcode=row, uops=lower(spec, ver=ver),
                      rd1_en=_has_src1(spec))
        sha[ver] = s.sha(ver)
    op = DveOp(name, spec, subdim=False, uops_sha=sha)
    dve_ops.OPS.append(op)
    dve_ops._SUB_OPCODE_FOR_NAME[name] = row
    dve_ops.CUSTOM_DVE_SPECS[name] = spec
    return op


# dir 1: h' = (x - vp)*0.75 + vp,  vp = sel(h < th1, h, 0)
_vp1 = select(Src1 < C1, Src1, Zero)
BILIF_STEP = _register(
    "BILIF_STEP",
    Spec(
        body=(Src0 - _vp1) * C0 + _vp1,
        reference=lambda in0, in1, s0, s1, imm2: (
            (in0 - np.where(in1 < s1, in1, 0).astype(np.float32))
            * np.float32(s0)
            + np.where(in1 < s1, in1, 0).astype(np.float32)
        ),
    ),
)

# dir 2, shifted state g = h2 - SHIFT (C2 = -SHIFT):
#   h2_prev = g_prev - C2;  vp = sel(g_prev < C1, g_prev - C2, 0)
#   g' = (x - vp)*C0 + vp + C2
_vp2 = select(Src1 < C1, Src1 - C2, Zero)
BILIF_STEP_S = _register(
    "BILIF_STEP_S",
    Spec(
        body=(Src0 - _vp2) * C0 + _vp2 + C2,
        reference=lambda in0, in1, s0, s1, imm2: (
            (in0 - np.where(in1 < s1, in1 - imm2, 0).astype(np.float32))
            * np.float32(s0)
            + np.where(in1 < s1, in1 - imm2, 0).astype(np.float32)
            + np.float32(imm2)
        ),
    ),
)


def _pack_weights() -> np.ndarray:
    """[128, 256] fp8e4m3: W_A = [:, :128] maps partition pair (2q, 2q+1)
    -> psum row q with weights (0.5, 1.5); W_B = [:, 128:] -> row 64+q.
    Other columns zero, so all matmuls can accumulate full-width."""
    w = np.zeros((128, 256), np.float32)
    for q in range(64):
        w[2 * q, q] = 0.5
        w[2 * q + 1, q] = 1.5
        w[2 * q, 128 + 64 + q] = 0.5
        w[2 * q + 1, 128 + 64 + q] = 1.5
    return w.astype(ml_dtypes.float8_e4m3)


_NC_CACHE = {}


def _build_nc(repeat: int = 1):
    """Build + compile the SPMD per-core program. `repeat` replays the body
    (used only for steady-state timing experiments)."""
    key = repeat
    if key in _NC_CACHE:
        return _NC_CACHE[key]
    nc = bacc.Bacc("TRN2", target_bir_lowering=False, debug=False,
                   num_devices=NCORES)
    x_d = nc.dram_tensor("x", [T * P, FREE], F32, kind="ExternalInput").ap()
    w_d = nc.dram_tensor("w", [P, 2 * P], FP8, kind="ExternalInput").ap()
    o_d = nc.dram_tensor("o", [T * P, FREE // 2], FP8,
                         kind="ExternalOutput").ap()

    with tile.TileContext(nc) as tc:
        with tc.tile_pool(name="xp", bufs=17) as xp, \
             tc.tile_pool(name="hp", bufs=3) as hp, \
             tc.tile_pool(name="ap", bufs=2) as apool, \
             tc.tile_pool(name="outp", bufs=4) as outp, \
             tc.tile_pool(name="psp", bufs=4, space="PSUM") as psp, \
             tc.tile_pool(name="zp", bufs=1) as zp:
            wa = zp.tile([P, P], FP8, tag="wa", name="wa")
            nc.sync.dma_start(out=wa[:], in_=w_d[:, :P])
            wb = zp.tile([P, P], FP8, tag="wb", name="wb")
            nc.sync.dma_start(out=wb[:], in_=w_d[:, P:])
            b1 = zp.tile([P, 1], F32, tag="b1", name="b1")
            nc.vector.memset(b1[:], -TH1)
            for rep in range(repeat):
                for k in range(NCHUNK):
                    c0 = k * CHUNK
                    # Load each x[t] tile once, in first-use order
                    # (fwd uses t at step t, bwd uses t at step 15-t).
                    xt = {}
                    for t in [v for s in range(T // 2) for v in (s, T - 1 - s)]:
                        xt[t] = xp.tile([P, CHUNK], F32, tag="x",
                                        name=f"x{rep}_{k}_{t}")
                        nc.sync.dma_start(
                            out=xt[t][:],
                            in_=x_d[t * P:(t + 1) * P, c0:c0 + CHUNK])
                    h_prev = None
                    pending = None  # (ps, t) awaiting copy+store
                    for t in range(T):
                        # h[:, :CHUNK] = h1;  h[:, CHUNK:] = g = h2 - SHIFT
                        h = hp.tile([P, 2 * CHUNK], F32, tag="h", name="h")
                        if t == 0:
                            # v = 0: h1 = .75x, g = .75x' - SHIFT -- on ACT
                            # (keeps the critical DVE chain 2 ops/step)
                            nc.scalar.activation(
                                out=h[:, :CHUNK], in_=xt[0][:],
                                func=AF.Copy, bias=0.0, scale=R)
                            nc.scalar.activation(
                                out=h[:, CHUNK:], in_=xt[T - 1][:],
                                func=AF.Copy, bias=-SHIFT, scale=R)
                        else:
                            nc.vector._custom_dve(
                                BILIF_STEP, out=h[:, :CHUNK], in0=xt[t][:],
                                in1=h_prev[:, :CHUNK], s0=R, s1=TH1)
                            nc.vector._custom_dve(
                                BILIF_STEP_S, out=h[:, CHUNK:],
                                in0=xt[T - 1 - t][:],
                                in1=h_prev[:, CHUNK:], s0=R, s1=TH1,
                                imm2=-SHIFT)
                        # Drain the previous step's psum first so the
                        # in-order ACT queue never waits on this step's DVE
                        if pending is not None:
                            _drain(nc, outp, o_d, pending, c0)
                        # One Sign for both dirs: sigma = sign(h - 0.75)
                        a = apool.tile([P, 2 * CHUNK], FP8, tag="a",
                                       name="a")
                        nc.scalar.activation(out=a[:], in_=h[:],
                                             func=AF.Sign, bias=b1[:],
                                             scale=1.0)
                        # Pack-combine: psum[q, f] = t[2q] + 3*t[2q+1] at
                        # data col f (strip A, rows 0:64) / 1024+f (strip
                        # B, rows 64:128), t = (sig1+sig2)/2.
                        ps = psp.tile([P, HALF], F32, tag="ps", name="ps")
                        for j in (0, 512):  # one PSUM bank (512 f32) each
                            po = slice(j, j + 512)
                            sa = slice(j, j + 512)
                            sb = slice(HALF + j, HALF + j + 512)
                            nc.tensor.matmul(ps[:, po], wa[:], a[:, sa],
                                             start=True, stop=False)
                            nc.tensor.matmul(ps[:, po], wa[:],
                                             a[:, CHUNK + j:CHUNK + j + 512],
                                             start=False, stop=False)
                            nc.tensor.matmul(ps[:, po], wb[:], a[:, sb],
                                             start=False, stop=False)
                            nc.tensor.matmul(
                                ps[:, po], wb[:],
                                a[:, CHUNK + HALF + j:CHUNK + HALF + j + 512],
                                start=False, stop=True)
                        pending = (ps, t)
                        h_prev = h
                    _drain(nc, outp, o_d, pending, c0)

    nc.compile()
    _NC_CACHE[key] = nc
    return nc


def _drain(nc, outp, o_d, pending, c0):
    """ACT copy psum -> fp8 (p in {-4..4} step .5: exact), then store."""
    ps, t = pending
    o = outp.tile([P, HALF], FP8, tag="o", name="o")
    nc.scalar.activation(out=o[:], in_=ps[:], func=AF.Copy,
                         bias=0.0, scale=1.0)
    nc.sync.dma_start(
        out=o_d[t * P:(t + 1) * P, c0 // 2:c0 // 2 + HALF], in_=o[:])


def _run(inputs: np.ndarray, repeat: int = 1, **kwargs):
    nc = _build_nc(repeat)
    w = _pack_weights()
    in_maps = []
    for c in range(NCORES):
        shard = np.ascontiguousarray(
            inputs[:, c * BS:(c + 1) * BS, :]).reshape(T * P, FREE)
        in_maps.append({"x": shard, "w": w})
    return bass_utils.run_bass_kernel_spmd(
        nc, in_maps, core_ids=list(range(NCORES)), **kwargs)


def _decode(o8: np.ndarray) -> np.ndarray:
    """[T*P, FREE//2] fp8 packed base-3 -> [T, BS, N] f32 output.

    Packed tile row q (resp. 64+q) col f of chunk k holds p = t0 + 3*t1
    for partitions (2q, 2q+1) at column k*2048 + f (resp. + 1024 + f),
    with t = (sig1+sig2)/2; out = (t+1)/2."""
    p = o8.astype(np.float32).reshape(T, P, NCHUNK, HALF)
    t1 = np.round(p / 3.0)
    t0 = p - 3.0 * t1
    out = np.empty((T, P, FREE), np.float32)
    for k in range(NCHUNK):
        for band, cols in ((0, slice(k * CHUNK, k * CHUNK + HALF)),
                           (64, slice(k * CHUNK + HALF, (k + 1) * CHUNK))):
            out[:, 0:P:2, cols] = (t0[:, band:band + 64, k, :] + 1.0) * 0.5
            out[:, 1:P:2, cols] = (t1[:, band:band + 64, k, :] + 1.0) * 0.5
    return out.reshape(T, BS, N)


def kernel(inputs: np.ndarray, **kwargs) -> np.ndarray:
    inputs = np.asarray(inputs)
    assert inputs.shape == (T, B, N) and inputs.dtype == np.float32
    res = None
    err = None
    for _attempt in range(3):  # retry transient device faults
        try:
            res = _run(inputs, **kwargs)
            break
        except Exception as e:  # noqa: BLE001
            err = e
    if res is None:
        raise err
    out = np.empty((T, B, N), np.float32)
    for c in range(NCORES):
        out[:, c * BS:(c + 1) * BS, :] = _decode(res.results[c]["o"])
    return out

